# revision 1
# baseline (speedup 1.0000x reference)
"""Causal self-attention with int8 KV quant-dequant on 8 Trainium2 cores.

Sharding: 8 cores = 4 batches x 2 head-groups (tensor parallel over heads).
Core c handles batch b=c//2, head-group g=c%2 (8 of 16 heads).
 - c_attn column-split per head-group; per-tensor K/V absmax all-reduced (max)
   across all 8 cores on-device.
 - c_proj row-split; the two per-batch partial outputs are summed on host.

All matmuls run in float32r (TF32-like: fp32 with 11-bit round-to-nearest-even
mantissa) at full PE rate. Attention computed in transposed score layout
scoresT[k, q] so softmax needs no transposes: exp on ACT, denominator via a
ones[128,1] matmul, normalization by a PE-replicated reciprocal row.
Softmax skips max-subtraction: |scores| <= ~10 here, exp is safe in fp32.
"""

import math

import numpy as np

N_HEAD = 16
B, T, C = 4, 2048, 2048
HS = C // N_HEAD  # 128
NCORES = 8
HPG = 8           # heads per group
CL = HPG * HS     # 1024 local feature dim
P = 128
TT = T // P       # 16 T-tiles
CT = C // P       # 16 C-tiles
NG = T // 512     # 4 q-groups of 512

_RUNNER = None
DEBUG = False


def _round_f32r(x: np.ndarray) -> np.ndarray:
    """Round fp32 to float32r precision: 11 mantissa bits, round-to-nearest-even."""
    b = np.ascontiguousarray(x, dtype=np.float32).view(np.uint32)
    shift = 12
    mask = np.uint32((1 << shift) - 1)
    half = np.uint32(1 << (shift - 1))
    frac = b & mask
    inc = (frac > half) | ((frac == half) & ((b >> shift) & np.uint32(1)).astype(bool))
    out = (b & ~mask) + np.where(inc, np.uint32(1 << shift), np.uint32(0))
    return out.view(np.float32)


def _split_sync_waits(nc):
    """Workaround for this walrus build: every instruction accepts only ONE
    sync-wait command. Hoist extra sem waits onto fresh same-engine NoOps
    inserted immediately before the instruction (engine streams are in-order,
    so all waits still complete before the instruction issues)."""
    import concourse.mybir as mybir

    n_split = 0
    for bb in nc.main_func.blocks:
        insts = bb.instructions
        i = 0
        while i < len(insts):
            inst = insts[i]
            si = getattr(inst, "sync_info", None)
            if si is not None and len(si.on_wait) > 1:
                waits = list(si.on_wait)
                eng = inst.engine
                nops = []
                for w in waits[:-1]:
                    nop = mybir.InstNoOp(
                        name=nc.get_next_instruction_name(),
                        engine=eng,
                        bass_nofuse=True,
                        sync_info=mybir.SyncInfo(on_wait=[w], on_update=[]),
                    )
                    nops.append(nop)
                inst.sync_info = mybir.SyncInfo(
                    on_wait=[waits[-1]], on_update=list(si.on_update)
                )
                insts[i:i] = nops
                i += len(nops)
                n_split += 1
            i += 1
    return n_split


def _build_nc():
    import concourse.bass as bass
    import concourse.mybir as mybir
    import concourse.tile as tile

    f32 = mybir.dt.float32
    f32r = mybir.dt.float32r
    i32 = mybir.dt.int32
    Alu = mybir.AluOpType
    Act = mybir.ActivationFunctionType

    nc = bass.Bass("TRN2", target_bir_lowering=False, debug=False,
                   num_devices=NCORES)

    xt_ap = nc.dram_tensor("xt", [C, T], f32r, kind="ExternalInput").ap()
    wq_ap = nc.dram_tensor("wq", [C, 3 * CL], f32r, kind="ExternalInput").ap()
    wp_ap = nc.dram_tensor("wp", [CL, C], f32r, kind="ExternalInput").ap()
    idr_ap = nc.dram_tensor("idr", [P, P], f32r, kind="ExternalInput").ap()
    idf_ap = nc.dram_tensor("idf", [P, P], f32, kind="ExternalInput").ap()
    maskT_ap = nc.dram_tensor("maskT", [P, P], f32, kind="ExternalInput").ap()
    part_ap = nc.dram_tensor("part", [T, C], f32, kind="ExternalOutput").ap()
    if DEBUG:
        dbg_qkvT_ap = nc.dram_tensor("dbg_qkvT", [3 * CL, T], f32,
                                     kind="ExternalOutput").ap()
        dbg_scpp_ap = nc.dram_tensor("dbg_scpp", [P, 4], f32,
                                     kind="ExternalOutput").ap()
        dbg_yt_ap = nc.dram_tensor("dbg_yt", [CL, T], f32,
                                   kind="ExternalOutput").ap()
        dbg_stats_ap = nc.dram_tensor("dbg_stats", [P, 64], f32,
                                      kind="ExternalOutput").ap()
        dbg_cm_ap = nc.dram_tensor("dbg_cm", [P, 2], f32, kind="ExternalOutput").ap()
        dbg_gm2_ap = nc.dram_tensor("dbg_gm2", [2, 1], f32, kind="ExternalOutput").ap()
        dbg_gmax_ap = nc.dram_tensor("dbg_gmax", [1, 2], f32, kind="ExternalOutput").ap()
        dbg_row4_ap = nc.dram_tensor("dbg_row4", [1, 4], f32, kind="ExternalOutput").ap()
        dbg_vals4_ap = nc.dram_tensor("dbg_vals4", [4, 1], f32, kind="ExternalOutput").ap()

    NF = 3 * CL // P  # 24 feature tiles (q:0-7, k:8-15, v:16-23)
    inv_sqrt_hs = float(1.0 / math.sqrt(HS))

    with tile.TileContext(nc) as tc:
        with (
            tc.tile_pool(name="persist", bufs=1) as persist,
            tc.tile_pool(name="dram", bufs=1, space="DRAM") as dram,
        ):
            qkvT = dram.tile([3 * CL, T], f32r)
            ytspill = dram.tile([CL, T], f32r)
            cc_in = dram.tile([1, 16], f32)
            cc_out = dram.tile([1, 16], f32)

            idr = persist.tile([P, P], f32r, name="idr_sb")
            nc.sync.dma_start(idr[:], idr_ap[:])
            idf = persist.tile([P, P], f32, name="idf_sb")
            nc.sync.dma_start(idf[:], idf_ap[:])
            maskT = persist.tile([P, P], f32, name="maskT_sb")
            nc.sync.dma_start(maskT[:], maskT_ap[:])
            ones_p1 = persist.tile([P, 1], f32r, name="ones_p1")
            nc.vector.memset(ones_p1[:].bitcast(f32), 1.0)
            ones_1r = persist.tile([1, P], f32r, name="ones_1r")
            nc.vector.memset(ones_1r[:].bitcast(f32), 1.0)
            stats = persist.tile([P, 64], f32, name="stats")
            scpp = persist.tile([P, 4], f32, name="scpp")  # sc_k, sc_v, inv_k, inv_v

            # ---------------- Phase 1: qkvT = (x @ Wqkv)^T + k/v absmax stats
            with (
                tc.tile_pool(name="xtp", bufs=1) as xtp,
                tc.tile_pool(name="wstrip", bufs=3) as wstrip,
                tc.tile_pool(name="p1ps", bufs=3, space="PSUM") as p1ps,
                tc.tile_pool(name="p1st", bufs=3) as p1st,
            ):
                xts = xtp.tile([P, CT, T], f32r, name="xts")
                for ct in range(CT):
                    nc.sync.dma_start(xts[:, ct, :], xt_ap[ct * P:(ct + 1) * P, :])
                for f in range(NF):
                    ws = wstrip.tile([P, CT, P], f32r, name="ws")
                    nc.sync.dma_start(
                        ws[:],
                        wq_ap[:, f * P:(f + 1) * P].rearrange(
                            "(ct p) m -> p ct m", p=P),
                    )
                    for g4 in range(NG):
                        ps = p1ps.tile([P, 512], f32, name="p1ps_t")
                        for ct in range(CT):
                            nc.tensor.matmul(
                                ps[:], ws[:, ct, :],
                                xts[:, ct, g4 * 512:(g4 + 1) * 512],
                                start=(ct == 0), stop=(ct == CT - 1),
                            )
                        st = p1st.tile([P, 512], f32r, name="p1st_t")
                        nc.scalar.copy(st[:], ps[:])
                        nc.sync.dma_start(
                            qkvT[f * P:(f + 1) * P, g4 * 512:(g4 + 1) * 512],
                            st[:],
                        )
                        if f >= 8:
                            nc.vector.tensor_reduce(
                                stats[:, (f - 8) * NG + g4:(f - 8) * NG + g4 + 1],
                                st[:], axis=mybir.AxisListType.X,
                                op=Alu.max, apply_absolute_value=True,
                            )

            # ---------------- Phase 2: global absmax + scales
            with (
                tc.tile_pool(name="p2", bufs=1) as p2,
                tc.tile_pool(name="p2ps", bufs=1, space="PSUM") as p2ps,
            ):
                # NB: PE transposes of tiny tiles (free dim < 32) silently
                # produce garbage on this HW -- always transpose padded 128x128.
                colmax = p2.tile([P, P], f32, name="colmax")
                nc.vector.memset(colmax[:], 0.0)
                nc.vector.tensor_reduce(colmax[:, 0:1], stats[:, 0:32],
                                        axis=mybir.AxisListType.X, op=Alu.max)
                nc.vector.tensor_reduce(colmax[:, 1:2], stats[:, 32:64],
                                        axis=mybir.AxisListType.X, op=Alu.max)
                pstat = p2ps.tile([P, P], f32, name="pstat")
                nc.tensor.transpose(pstat[:], colmax[:], idf[:])
                gm2 = p2.tile([2, 1], f32, name="gm2")
                nc.vector.tensor_reduce(gm2[:], pstat[0:2, :],
                                        axis=mybir.AxisListType.X, op=Alu.max)
                # [2,1] -> row [1,16] via padded PE transpose (no cross-partition DMA)
                gm_pad = p2.tile([P, P], f32, name="gm_pad")
                nc.vector.memset(gm_pad[:], 0.0)
                nc.vector.tensor_copy(gm_pad[0:2, 0:1], gm2[:])
                pgm = p2ps.tile([P, P], f32, name="pgm")
                nc.tensor.transpose(pgm[:], gm_pad[:], idf[:])
                ccrow = p2.tile([1, 16], f32, name="ccrow")
                nc.vector.tensor_copy(ccrow[:], pgm[0:1, 0:16])
                nc.sync.dma_start(cc_in[:], ccrow[:])
                nc.gpsimd.collective_compute(
                    "AllReduce", Alu.max,
                    replica_groups=[list(range(NCORES))],
                    ins=[cc_in.opt()], outs=[cc_out.opt()],
                )
                gmax_row = p2.tile([1, 16], f32, name="gmax_row")
                nc.sync.dma_start(gmax_row[:], cc_out[:])
                gmax = gmax_row[:, 0:2]
                row4 = p2.tile([1, 4], f32, name="row4")
                recip2 = p2.tile([1, 2], f32, name="recip2")
                nc.vector.reciprocal(recip2[:], gmax)
                nc.vector.tensor_scalar(row4[:, 0:2], gmax, 1.0 / 127.0, None,
                                        op0=Alu.mult)
                nc.vector.tensor_scalar(row4[:, 2:4], recip2[:], 127.0, None,
                                        op0=Alu.mult)
                # [1,4] -> [4,1] via padded PE transpose, then broadcast rows
                row_pad = p2.tile([P, P], f32, name="row_pad")
                nc.vector.memset(row_pad[:], 0.0)
                nc.vector.tensor_copy(row_pad[0:1, 0:4], row4[:])
                prow = p2ps.tile([P, P], f32, name="prow")
                nc.tensor.transpose(prow[:], row_pad[:], idf[:])
                vals4 = p2.tile([4, 1], f32, name="vals4")
                nc.vector.tensor_copy(vals4[:], prow[0:4, 0:1])
                ones4 = p2.tile([4, P], f32, name="ones4")
                nc.vector.memset(ones4[:], 1.0)
                rows_pad = p2.tile([P, P], f32, name="rows_pad")
                nc.vector.memset(rows_pad[:], 0.0)
                nc.vector.tensor_scalar(rows_pad[0:4, :], ones4[:], vals4[:], None,
                                        op0=Alu.mult)
                prr = p2ps.tile([P, P], f32, name="prr")
                nc.tensor.transpose(prr[:], rows_pad[:], idf[:])
                nc.vector.tensor_copy(scpp[:], prr[:, 0:4])
                if DEBUG:
                    nc.sync.dma_start(dbg_cm_ap[:], colmax[:, 0:2])
                    nc.sync.dma_start(dbg_gm2_ap[:], gm2[:])
                    nc.sync.dma_start(dbg_gmax_ap[:], gmax[:])
                    nc.sync.dma_start(dbg_row4_ap[:], row4[:])
                    nc.sync.dma_start(dbg_vals4_ap[:], vals4[:])

            # ---------------- Phase 3: attention per head
            with (
                tc.tile_pool(name="hd", bufs=2) as hd,
                tc.tile_pool(name="hq", bufs=2) as hq,
                tc.tile_pool(name="ex", bufs=4) as exp_pool,
                tc.tile_pool(name="nrm", bufs=2) as nrm,
                tc.tile_pool(name="yth", bufs=2) as yth_pool,
                tc.tile_pool(name="ps_s", bufs=3, space="PSUM") as ps_s,
                tc.tile_pool(name="ps_o", bufs=2, space="PSUM") as ps_o,
                tc.tile_pool(name="ps_d", bufs=2, space="PSUM") as ps_d,
            ):
                for h in range(HPG):
                    yth = yth_pool.tile([P, T], f32r, name="yth", tag="yth")
                    qT = hd.tile([P, T], f32r, name="qT", tag="qT")
                    nc.sync.dma_start(qT[:], qkvT[h * P:(h + 1) * P, :])
                    kraw = hd.tile([P, T], f32r, name="kraw", tag="kraw")
                    nc.sync.dma_start(kraw[:],
                                      qkvT[CL + h * P:CL + (h + 1) * P, :])
                    vraw = hd.tile([P, T], f32r, name="vraw", tag="vraw")
                    nc.sync.dma_start(vraw[:],
                                      qkvT[2 * CL + h * P:2 * CL + (h + 1) * P, :])

                    kT = hd.tile([P, T], f32r, name="kT", tag="kT")
                    vT = hd.tile([P, T], f32r, name="vT", tag="vT")
                    for (raw, dq, ci) in ((kraw, kT, 0), (vraw, vT, 1)):
                        tmp = hq.tile([P, T], f32, name="tmp", tag="qtmp")
                        nc.vector.tensor_scalar(tmp[:], raw[:],
                                                scpp[:, 2 + ci:3 + ci], None,
                                                op0=Alu.mult)
                        nc.vector.tensor_scalar(tmp[:], tmp[:], 127.0, -127.0,
                                                op0=Alu.min, op1=Alu.max)
                        tmpi = hq.tile([P, T], i32, name="tmpi", tag="qtmpi")
                        nc.vector.tensor_copy(tmpi[:], tmp[:])
                        nc.vector.tensor_scalar(dq[:], tmpi[:],
                                                scpp[:, ci:ci + 1], None,
                                                op0=Alu.mult)

                    vN = hd.tile([P, TT, P], f32r, name="vN", tag="vN")
                    for kt in range(TT):
                        pt = ps_s.tile([P, 512], f32r, name="ptr", tag="ps_s")
                        nc.tensor.transpose(pt[:, 0:P],
                                            vT[:, kt * P:(kt + 1) * P], idr[:])
                        nc.vector.tensor_copy(vN[:, kt, :], pt[:, 0:P])

                    for gq in range(NG):
                        kmax_t = 4 * gq + 3
                        po = ps_o.tile([P, 512], f32, name="po", tag="po")
                        pd = ps_d.tile([1, 512], f32, name="pd", tag="pd")
                        for ki in range(kmax_t + 1):
                            off = max(0, ki * P - gq * 512)
                            ps = ps_s.tile([P, 512], f32, name="ps", tag="ps_s")
                            nc.tensor.matmul(
                                ps[:, off:], kT[:, ki * P:(ki + 1) * P],
                                qT[:, gq * 512 + off:(gq + 1) * 512],
                                start=True, stop=True,
                            )
                            ex = exp_pool.tile([P, 512], f32r, name="ex", tag="ex")
                            nc.scalar.activation(ex[:, off:], ps[:, off:],
                                                 Act.Exp, scale=inv_sqrt_hs)
                            if ki >= 4 * gq:
                                nc.vector.tensor_tensor(
                                    ex[:, off:off + P], ex[:, off:off + P],
                                    maskT[:], Alu.mult)
                            nc.tensor.matmul(po[:, off:], vN[:, ki, :],
                                             ex[:, off:],
                                             start=(ki == 0), stop=(ki == kmax_t))
                            nc.tensor.matmul(pd[:, off:], ones_p1[:],
                                             ex[:, off:],
                                             start=(ki == 0), stop=(ki == kmax_t))
                        rrow = nrm.tile([1, 512], f32, name="rrow", tag="rrow")
                        nc.vector.reciprocal(rrow[:], pd[0:1, :])
                        rrowr = nrm.tile([1, 512], f32r, name="rrowr", tag="rrowr")
                        nc.vector.tensor_copy(rrowr[:], rrow[:])
                        pr = ps_s.tile([P, 512], f32, name="pr", tag="ps_s")
                        nc.tensor.matmul(pr[:], ones_1r[:], rrowr[:],
                                         start=True, stop=True)
                        rep = nrm.tile([P, 512], f32, name="rep", tag="rep")
                        nc.scalar.copy(rep[:], pr[:])
                        nc.vector.tensor_tensor(
                            yth[:, gq * 512:(gq + 1) * 512],
                            po[:], rep[:], Alu.mult)
                    nc.sync.dma_start(ytspill[h * P:(h + 1) * P, :], yth[:])

            if DEBUG:
                nc.sync.dma_start(dbg_stats_ap[:], stats[:])
                nc.sync.dma_start(dbg_qkvT_ap[:], qkvT[:].bitcast(f32))
                nc.sync.dma_start(dbg_scpp_ap[:], scpp[:])
                nc.sync.dma_start(dbg_yt_ap[:], ytspill[:].bitcast(f32))

            # ---------------- Phase 4: out = y @ Wproj (partial)
            with (
                tc.tile_pool(name="wpp", bufs=1) as wpp,
                tc.tile_pool(name="p4st", bufs=4) as p4st,
                tc.tile_pool(name="p4ps", bufs=8, space="PSUM") as p4ps,
            ):
                wps = wpp.tile([P, HPG, C], f32r, name="wps")
                yres = wpp.tile([P, HPG, T], f32r, name="yres")
                for ci in range(HPG):
                    nc.sync.dma_start(wps[:, ci, :],
                                      wp_ap[ci * P:(ci + 1) * P, :])
                    nc.sync.dma_start(yres[:, ci, :],
                                      ytspill[ci * P:(ci + 1) * P, :])
                for tch in range(4):
                    for n in range(NG):
                        pts = [p4ps.tile([P, 512], f32, name=f"p4_{t}",
                                         tag="p4ps") for t in range(4)]
                        for ci in range(HPG):
                            for t in range(4):
                                tt = tch * 4 + t
                                nc.tensor.matmul(
                                    pts[t][:],
                                    yres[:, ci, tt * P:(tt + 1) * P],
                                    wps[:, ci, n * 512:(n + 1) * 512],
                                    start=(ci == 0), stop=(ci == HPG - 1),
                                )
                        for t in range(4):
                            tt = tch * 4 + t
                            ot = p4st.tile([P, 512], f32, name="ot", tag="ot")
                            nc.scalar.copy(ot[:], pts[t][:])
                            nc.sync.dma_start(
                                part_ap[tt * P:(tt + 1) * P,
                                        n * 512:(n + 1) * 512],
                                ot[:],
                            )

    _split_sync_waits(nc)
    return nc


def _shard_inputs(x, W_attn, W_proj):
    x = np.asarray(x, dtype=np.float32)
    W_attn = np.asarray(W_attn, dtype=np.float32)
    W_proj = np.asarray(W_proj, dtype=np.float32)

    idr = np.eye(P, dtype=np.float32)
    idf = np.eye(P, dtype=np.float32)
    kk, qq = np.meshgrid(np.arange(P), np.arange(P), indexing="ij")
    maskT = (kk <= qq).astype(np.float32)  # maskT[k_local, q_local]

    # cores 2b and 2b+1 share xt; head-groups share wq/wp -- compute each once
    xts = [np.ascontiguousarray(_round_f32r(x[b].T)) for b in range(B)]
    wqs = [np.ascontiguousarray(_round_f32r(np.concatenate([
        W_attn[:, g * CL:(g + 1) * CL],
        W_attn[:, C + g * CL:C + (g + 1) * CL],
        W_attn[:, 2 * C + g * CL:2 * C + (g + 1) * CL],
    ], axis=1))) for g in range(2)]
    wps = [np.ascontiguousarray(_round_f32r(W_proj[g * CL:(g + 1) * CL, :]))
           for g in range(2)]
    in_maps = []
    for c in range(NCORES):
        b, g = c // 2, c % 2
        in_maps.append({
            "xt": xts[b], "wq": wqs[g], "wp": wps[g],
            "idr": idr, "idf": idf, "maskT": maskT,
        })
    return in_maps


def _wait_device_healthy(max_tries=12, sleep_s=15):
    import time

    import jax
    import jax.numpy as jnp

    for i in range(max_tries):
        try:
            a = jnp.ones((8, 8))
            if float((a @ a).sum()) == 512.0:
                return
        except Exception:
            pass
        time.sleep(sleep_s)


class _Runner:
    """Compile the SPMD bass program once; reuse the jitted executable."""

    def __init__(self):
        import jax
        import numpy as _np
        import concourse.mybir as mybir
        from concourse import bass2jax
        from concourse.bass2jax import (
            _bass_exec_p,
            install_neuronx_cc_hook,
            partition_id_tensor,
        )
        from jax.sharding import Mesh, PartitionSpec
        from jax.experimental.shard_map import shard_map

        install_neuronx_cc_hook()
        nc = _build_nc()
        self.nc = nc

        partition_name = (nc.partition_id_tensor.name
                          if nc.partition_id_tensor else None)
        in_names, out_names, out_avals, zero_shapes = [], [], [], []
        for alloc in nc.m.functions[0].allocations:
            if not isinstance(alloc, mybir.MemoryLocationSet):
                continue
            name = alloc.memorylocations[0].name
            if alloc.kind == "ExternalInput":
                if name != partition_name:
                    in_names.append(name)
            elif alloc.kind == "ExternalOutput":
                shape = tuple(alloc.tensor_shape)
                dtype = mybir.dt.np(alloc.dtype)
                out_names.append(name)
                out_avals.append(jax.core.ShapedArray(shape, dtype))
                zero_shapes.append((shape, dtype))
        n_params = len(in_names)
        self.in_names = in_names
        self.out_names = out_names
        self.out_avals = out_avals
        self.zero_shapes = zero_shapes
        self.n_params = n_params

        all_names = list(in_names) + list(out_names)
        if partition_name is not None:
            all_names.append(partition_name)
        donate = tuple(range(n_params, n_params + len(out_names)))

        def _body(*args):
            operands = list(args)
            if partition_name is not None:
                operands.append(partition_id_tensor())
            outs = _bass_exec_p.bind(
                *operands,
                out_avals=tuple(out_avals),
                in_names=tuple(all_names),
                out_names=tuple(out_names),
                lowering_input_output_aliases=(),
                sim_require_finite=True,
                sim_require_nnan=True,
                nc=nc,
            )
            return tuple(outs)

        devices = jax.devices()[:NCORES]
        assert len(devices) == NCORES
        self.mesh = Mesh(_np.asarray(devices), ("core",))
        in_specs = (PartitionSpec("core"),) * (n_params + len(out_names))
        out_specs = (PartitionSpec("core"),) * len(out_names)
        self.sharded = jax.jit(
            shard_map(_body, mesh=self.mesh, in_specs=in_specs,
                      out_specs=out_specs, check_rep=False),
            donate_argnums=donate, keep_unused=True,
        )

    def concat_inputs(self, in_maps):
        return [
            np.concatenate([np.asarray(in_maps[c][nm]) for c in range(NCORES)],
                           axis=0)
            for nm in self.in_names
        ]

    def make_zeros(self):
        return [np.zeros((NCORES * s[0], *s[1:]), dt)
                for (s, dt) in self.zero_shapes]

    def execute(self, concat_in, zeros):
        out_arrs = self.sharded(*concat_in, *zeros)
        return out_arrs

    def run(self, in_maps):
        out_arrs = self.execute(self.concat_inputs(in_maps), self.make_zeros())
        results = []
        for c in range(NCORES):
            d = {}
            for i, nm in enumerate(self.out_names):
                full = np.asarray(out_arrs[i])
                per = self.out_avals[i].shape
                d[nm] = full.reshape(NCORES, *per)[c]
            results.append(d)
        return results


_RUNNER_OBJ = None


def _get_runner():
    global _RUNNER, _RUNNER_OBJ
    if _RUNNER is None:
        _wait_device_healthy()
        r = _Runner()
        _RUNNER_OBJ = r
        _RUNNER = r.run
    return _RUNNER


def kernel(x, W_attn, W_proj):
    run = _get_runner()
    in_maps = _shard_inputs(x, W_attn, W_proj)
    results = run(in_maps)
    out = np.empty((B, T, C), dtype=np.float32)
    for b in range(B):
        out[b] = results[2 * b]["part"] + results[2 * b + 1]["part"]
    return out


if __name__ == "__main__":
    rng = np.random.default_rng(0)
    x = rng.standard_normal((B, T, C)).astype(np.float32)
    Wa = (rng.standard_normal((C, 3 * C)) * 0.02).astype(np.float32)
    Wp = (rng.standard_normal((C, C)) * 0.02).astype(np.float32)
    out = kernel(x=x, W_attn=Wa, W_proj=Wp)
    print("kernel ran, out shape", out.shape, "mean", float(np.abs(out).mean()))

